# revision 28
# baseline (speedup 1.0000x reference)
"""Trainium2 Bass kernel for the Dale CB-cell step.

Math (per batch column b, H=48, IN=8):
    v      = hidden[b, :]                    (carried state)
    r      = sigmoid(v)
    zpre   = Ksp @ r + P_z @ x[:, b] + b_z
    u      = DT*(W @ r + P_masked @ x[:, b] + b_v)
    v_new  = v * (1 - DT*sigmoid(zpre)) + u

Sharding: pure batch data-parallel — each of the 8 cores gets 131072
rows of `hidden`/`x`; all weights are host-folded (softplus, Dale
masks, DT scaling, biases-via-ones-row) into one constant (64, 96)
bf16 matrix and replicated.

Active design (`_build_nc_v4`, used by `kernel()`): this backend
charges ~40-80us per UNIQUE instruction per run (program-size-driven;
superlinear), while For_i hardware-loop trip counts are nearly free
(measured: 12 vs 72 trips of an 8-matmul body cost the same wall).
v4 therefore wraps v2's proven dataflow in a For_i loop over
16384-row chunks with ds() dynamic DRAM offsets: per trip 1 hv load,
1 pre-transpose x-inject (batch-major x.T slab on the second HWDGE
queue, contiguous 128-partition pattern — ~0.9ms faster than the
legacy 16-partition scattered post-transpose inject), 1 ACT sigmoid,
1 DMA-xbar transpose to H-major, a nested For_i over 2048-column
windows (4 matmuls + 1 DVE psum eviction), 1 DMA-xbar transpose
back, a 4-op f32 epilogue and 1 store — ~25 unique instructions
total vs ~486 unrolled in v2, cutting the measured per-rep delta
from ~35.8ms to ~4.7ms (run-to-run drift 3.6-6.7ms; A/B only valid
back-to-back in one process). Inner-loop granularity 512 is WORSE
(inner iterations cost ~5-9us); 2048 is the sweet spot. Outer trips
cost ~342us each at chunk=16384; chunk=32768 does not fit SBUF. A
two-stream overlap variant (`_build_nc_v5`) LOST to v4 (extra unique
instructions outweigh overlap). `_build_nc` (v1) and `_build_nc_v2`
are kept for ablation only.
"""

import sys

if "/opt/trn_rl_repo" not in sys.path:
    sys.path.insert(0, "/opt/trn_rl_repo")

import numpy as np

H = 48
IN = 8
DT = 0.1
B = 1048576
N_CORES = 8
B_CORE = B // N_CORES          # 131072
MACRO = 2048                   # batch rows per macro-tile
N_SUB = MACRO // 128           # 16 subtiles per macro
N_CHUNK = N_SUB // 2           # 8 matmul chunks (2 subtiles each)
GQ = 256                       # psum column stride per chunk (bank-safe)

_NC_CACHE = {}


def _softplus64(x):
    x = x.astype(np.float64)
    return np.log1p(np.exp(-np.abs(x))) + np.maximum(x, 0.0)


def _build_rhs(P, b_v, K, C, P_z, b_z, e_e, e_i):
    """Host fold of all weights into the (128, 192) matmul rhs."""
    Ksp = _softplus64(K)
    Csp = _softplus64(C)
    S = Ksp + Csp
    e_e = float(np.asarray(e_e).reshape(-1)[0])
    e_i = float(np.asarray(e_i).reshape(-1)[0])
    W_E = np.maximum(e_e * S[:, : H // 2], 0.0)
    W_I = -np.maximum(-(e_i * S[:, H // 2 :]), 0.0)
    W = np.concatenate([W_E, W_I], axis=1)          # (H, H)
    rows = np.arange(H)
    keep = ~(((rows >= H // 4) & (rows < H // 2)) | (rows >= 3 * H // 4))
    P_masked = P.astype(np.float64) * keep[:, None]

    blk = np.zeros((64, 96), np.float64)
    blk[0:H, 0:H] = Ksp.T                     # z-half:  Ksp @ r
    blk[0:H, H : 2 * H] = (DT * W).T          # u-half:  DT * W @ r
    blk[H : H + IN, 0:H] = P_z.astype(np.float64).T
    blk[H : H + IN, H : 2 * H] = (DT * P_masked).T
    blk[H + IN, 0:H] = b_z.astype(np.float64).reshape(-1)
    blk[H + IN, H : 2 * H] = DT * b_v.astype(np.float64).reshape(-1)
    rhs = np.zeros((128, 192), np.float64)
    rhs[0:64, 0:96] = blk                     # even subtile rows
    rhs[64:128, 96:192] = blk                 # odd subtile rows
    return rhs


def _build_xpad(x):
    """(16, B) = [x; ones; zeros] permuted to the device batch layout.

    Device reads xpad[i, t0 + 256*cc + 128*a + e] as the x row for batch
    index t0 + 16*e + 2*cc + a.
    """
    xz = np.zeros((16, x.shape[1]), np.float32)
    xz[0:IN] = x
    xz[IN] = 1.0
    v = xz.reshape(16, -1, 128, 16)           # [i, m, e, s]
    v = v.reshape(16, v.shape[1], 128, 8, 2)  # [i, m, e, cc, a]
    w = np.ascontiguousarray(v.transpose(0, 1, 3, 4, 2))  # [i, m, cc, a, e]
    return w.reshape(16, x.shape[1])


def _build_nc(b_core, reps=1, stage=5, bench=False):
    """reps>1 repeats the whole body in one NEFF (for delta-timing).

    stage: ablation ladder for bottleneck isolation (5 = full kernel):
      0 DMA only (hv load + x injects + store hv)
      1 + ACT r-sigmoid + DMA-transpose (consumed via tiny scratch store)
      2 + matmuls (psum slice consumed via tiny scratch store)
      3 + ACT z-sigmoid (zs slice consumed via tiny scratch store)
      4 + DVE g/v_term (vt stored as output)
      5 full
    """
    import concourse.bacc as bacc
    import concourse.mybir as mybir
    import concourse.tile as tile

    F32 = mybir.dt.float32
    BF16 = mybir.dt.bfloat16
    SIG = mybir.ActivationFunctionType.Sigmoid

    n_macro = b_core // MACRO
    nc = bacc.Bacc("TRN2", target_bir_lowering=False, debug=False)
    # bench mode: big tensors are device-internal (uninitialized) so runs
    # carry no host<->device transfer; timing-only, results meaningless.
    big = "Internal" if bench else None
    hid = nc.dram_tensor("hidden", [b_core, H], F32, kind=big or "ExternalInput")
    xpad = nc.dram_tensor("xpad", [16, b_core], BF16, kind=big or "ExternalInput")
    rhsw = nc.dram_tensor("rhsw", [128, 192], BF16, kind="ExternalInput")
    out = nc.dram_tensor("out", [b_core, H], F32, kind=big or "ExternalOutput")
    dbg = nc.dram_tensor("dbg", [128, 64], F32, kind="ExternalOutput") if bench else None
    scratch = (
        nc.dram_tensor("scratch", [128, 256], F32) if stage in (2, 3) else None
    )
    scratchb = (
        nc.dram_tensor("scratchb", [128, 64], BF16) if stage in (0, 1) else None
    )

    FW = N_SUB * H                            # 768  f32 working width
    RW = N_SUB * 64                           # 1024 bf16 padded width

    with tile.TileContext(nc) as tc:
        with (
            tc.tile_pool(name="const", bufs=1) as cpool,
            tc.tile_pool(name="io", bufs=3) as iopool,
            tc.tile_pool(name="work", bufs=2) as wpool,
            tc.tile_pool(name="psum", bufs=2, space="PSUM") as ppool,
        ):
            rhs_sb = cpool.tile([128, 192], BF16)
            nc.sync.dma_start(rhs_sb[:], rhsw[:])

            # r staging buffers are manually double-buffered so their pad
            # columns can be zeroed exactly once (pool slot rotation would
            # leave junk/NaN bits there for the DMA transpose to read).
            rbm_bufs = [
                cpool.tile([128, RW], BF16, name=f"rbm{i}", tag=f"rbm{i}")
                for i in range(2)
            ]
            for rb in rbm_bufs:
                pad = rb[:].rearrange("p (c e) -> p c e", e=64)[:, :, H:64]
                nc.gpsimd.memset(pad, 0.0)

            for m in range(n_macro * reps):
                t0 = (m % n_macro) * MACRO

                hv = iopool.tile([128, FW], F32, tag="hv")
                hv3 = hv[:].rearrange("p (c h) -> p c h", h=H)
                nc.sync.dma_start(
                    hv3, hid[t0 : t0 + MACRO, :].rearrange("(p c) h -> p c h", c=N_SUB)
                )

                lhsT = wpool.tile([128, RW], BF16, tag="lhsT")
                if stage >= 1:
                    # r = sigmoid(v), bf16, in 64-col padded blocks
                    rbm = rbm_bufs[m % 2]
                    rb3 = rbm[:].rearrange("p (c e) -> p c e", e=64)[:, :, 0:H]
                    nc.scalar.activation(rb3, hv3, SIG)

                    # H-major activations: chunk cc of lhsT = transpose of
                    # rbm cols [128cc, 128cc+128)
                    lt3 = lhsT[:].rearrange("p (c e) -> p c e", e=128)
                    nc.sync.dma_start(lt3, rbm[:], transpose=True)

                # x/ones/zeros into the pad partitions
                xsrc = xpad[:, t0 : t0 + MACRO].rearrange("i (c e) -> i c e", e=256)
                nc.sync.dma_start(
                    lhsT[48:64, :].rearrange("p (c e) -> p c e", e=128),
                    xsrc[:, :, 0:128],
                )
                nc.sync.dma_start(
                    lhsT[112:128, :].rearrange("p (c e) -> p c e", e=128),
                    xsrc[:, :, 128:256],
                )
                if stage <= 1:
                    nc.sync.dma_start(scratchb[:, 0:64], lhsT[:, 0:64])
                    nc.sync.dma_start(
                        out[t0 : t0 + MACRO, :].rearrange("(p c) h -> p c h", c=N_SUB),
                        hv3,
                    )
                    continue

                ps = ppool.tile([128, N_CHUNK * GQ], F32, tag="ps")
                for cc in range(N_CHUNK):
                    nc.tensor.matmul(
                        ps[:, GQ * cc : GQ * cc + 192],
                        lhsT[:, 128 * cc : 128 * cc + 128],
                        rhs_sb[:],
                        start=True,
                        stop=True,
                    )
                if stage == 2:
                    tmp = wpool.tile([128, 64], F32, tag="pscopy")
                    nc.scalar.activation(
                        tmp[:], ps[:, 0:64], mybir.ActivationFunctionType.Copy
                    )
                    nc.sync.dma_start(scratch[:, 0:64], tmp[:])
                    nc.sync.dma_start(
                        out[t0 : t0 + MACRO, :].rearrange("(p c) h -> p c h", c=N_SUB),
                        hv3,
                    )
                    continue

                ps4 = (
                    ps[:]
                    .rearrange("p (g q) -> p g q", q=GQ)[:, :, 0:192]
                    .rearrange("p g (a x) -> p g a x", x=96)
                )
                ps_z = ps4[:, :, :, 0:H]
                ps_u = ps4[:, :, :, H : 2 * H]

                zs = wpool.tile([128, FW], F32, tag="zs")
                zs4 = zs[:].rearrange("p (g a x) -> p g a x", g=N_CHUNK, a=2)
                nc.scalar.activation(zs4, ps_z, SIG)
                if stage == 3:
                    nc.sync.dma_start(scratch[:, 0:64], zs[:, 0:64])
                    nc.sync.dma_start(
                        out[t0 : t0 + MACRO, :].rearrange("(p c) h -> p c h", c=N_SUB),
                        hv3,
                    )
                    continue

                gt = wpool.tile([128, FW], F32, tag="gt")
                nc.vector.tensor_scalar(
                    gt[:], zs[:], -DT, 1.0, mybir.AluOpType.mult, mybir.AluOpType.add
                )

                vt = wpool.tile([128, FW], F32, tag="vt")
                nc.vector.tensor_mul(vt[:], hv[:], gt[:])
                if stage == 4:
                    nc.sync.dma_start(
                        out[t0 : t0 + MACRO, :].rearrange("(p c) h -> p c h", c=N_SUB),
                        vt[:].rearrange("p (c h) -> p c h", h=H),
                    )
                    continue

                ot = iopool.tile([128, FW], F32, tag="ot")
                ot4 = ot[:].rearrange("p (g a x) -> p g a x", g=N_CHUNK, a=2)
                vt4 = vt[:].rearrange("p (g a x) -> p g a x", g=N_CHUNK, a=2)
                nc.vector.tensor_add(ot4, vt4, ps_u)

                nc.sync.dma_start(
                    out[t0 : t0 + MACRO, :].rearrange("(p c) h -> p c h", c=N_SUB),
                    ot[:].rearrange("p (c h) -> p c h", h=H),
                )

            if bench:
                dbg_t = cpool.tile([128, 64], F32, name="dbg_t", tag="dbg_t")
                nc.gpsimd.memset(dbg_t[:], 0.0)
                nc.sync.dma_start(dbg[:], dbg_t[:])

    nc.compile()
    return nc


MACRO2 = 8192                 # v2 macro-tile rows
N_SUB2 = MACRO2 // 128        # 64 subtiles (one per 128-col transpose chunk)
N_WIN = MACRO2 // 512         # 16 matmul windows per macro


def _build_nc_v2(b_core, reps=1, bench=False, mm_n=512):
    """v2: instruction-count-minimized design.

    Per 8192-row macro: 1 hv load, 1 sigmoid, 1 DMA-transpose (1 subtile
    per 128-chunk, K rows 0:64 = [r(48); x/ones/zeros(16)]), 1 x-inject,
    16 matmuls (lhsT = constant (64, 96) weights, rhs = 512 batch cols),
    4 DVE psum evictions to bf16, 1 DMA-transpose back to batch-major,
    then a 4-op in-place f32 epilogue + 1 store.
    """
    import concourse.bacc as bacc
    import concourse.mybir as mybir
    import concourse.tile as tile

    F32 = mybir.dt.float32
    BF16 = mybir.dt.bfloat16
    SIG = mybir.ActivationFunctionType.Sigmoid

    n_macro = b_core // MACRO2
    nc = bacc.Bacc("TRN2", target_bir_lowering=False, debug=False)
    big = "Internal" if bench else None
    hid = nc.dram_tensor("hidden", [b_core, H], F32, kind=big or "ExternalInput")
    xpad = nc.dram_tensor("xpad", [16, b_core], BF16, kind=big or "ExternalInput")
    rhsw = nc.dram_tensor("rhsw", [64, 96], BF16, kind="ExternalInput")
    out = nc.dram_tensor("out", [b_core, H], F32, kind=big or "ExternalOutput")
    dbg = nc.dram_tensor("dbg", [128, 64], F32, kind="ExternalOutput") if bench else None

    FW = N_SUB2 * H               # 3072  (f32 working width per macro)
    RW = N_SUB2 * 128             # 8192  (bf16 padded width per macro)

    with tile.TileContext(nc) as tc:
        with (
            tc.tile_pool(name="const", bufs=1) as cpool,
            tc.tile_pool(name="io", bufs=2) as iopool,
            tc.tile_pool(name="work", bufs=2) as wpool,
            tc.tile_pool(name="psum", bufs=2, space="PSUM") as ppool,
        ):
            w_sb = cpool.tile([64, 96], BF16)
            nc.sync.dma_start(w_sb[:], rhsw[:])

            # manual double-buffers for the two transpose sources so their
            # never-written pad regions can be zeroed exactly once (keeps
            # CoreSim's uninit-read check green; HW wouldn't care).
            rbm_bufs = [
                cpool.tile([128, RW], BF16, name=f"rbm2_{i}", tag=f"rbm2_{i}")
                for i in range(2)
            ]
            for rb in rbm_bufs:
                pad = rb[:].rearrange("p (c e) -> p c e", e=128)[:, :, H:128]
                nc.gpsimd.memset(pad, 0.0)
            zu_bufs = [
                cpool.tile([128, RW], BF16, name=f"zu2_{i}", tag=f"zu2_{i}")
                for i in range(2)
            ]
            for zb in zu_bufs:
                nc.gpsimd.memset(zb[96:128, :], 0.0)

            for m in range(n_macro * reps):
                t0 = (m % n_macro) * MACRO2

                # batch(p, c) = t0 + 64*p + c, c in [0, 64)
                hv = iopool.tile([128, FW], F32, tag="hv", bufs=3)
                hv3 = hv[:].rearrange("p (c h) -> p c h", h=H)
                nc.sync.dma_start(
                    hv3,
                    hid[t0 : t0 + MACRO2, :].rearrange("(p c) h -> p c h", c=N_SUB2),
                )

                # r = sigmoid(v) bf16 into 128-col padded chunks
                rbm = rbm_bufs[m % 2]
                rb3 = rbm[:].rearrange("p (c e) -> p c e", e=128)[:, :, 0:H]
                nc.scalar.activation(rb3, hv3, SIG)

                # chunk c of trans = transpose of rbm cols [128c, 128c+128):
                # rows 0:48 = r (H-major), 48:64 <- x/ones/zeros, 64:128 junk
                trans = wpool.tile([128, RW], BF16, tag="trans")
                tr3 = trans[:].rearrange("p (c e) -> p c e", e=128)
                nc.sync.dma_start(tr3, rbm[:], transpose=True)
                nc.sync.dma_start(
                    trans[48:64, :].rearrange("p (c e) -> p c e", e=128),
                    xpad[:, t0 : t0 + MACRO2].rearrange("i (c e) -> i c e", e=128),
                )

                # zu: cols 128c+e <-> batch(e, c); rows [z(48) | u(48)]
                zu = zu_bufs[m % 2]
                mm_per_ps = 2048 // mm_n
                for g in range(4):
                    ps = ppool.tile([96, 2048], F32, tag="ps")
                    for s in range(mm_per_ps):
                        w = mm_per_ps * g + s
                        nc.tensor.matmul(
                            ps[:, mm_n * s : mm_n * s + mm_n],
                            w_sb[:],
                            trans[0:64, mm_n * w : mm_n * w + mm_n],
                            start=True,
                            stop=True,
                        )
                    nc.vector.tensor_copy(
                        zu[0:96, 2048 * g : 2048 * g + 2048], ps[:]
                    )

                # back to batch-major: zuT chunk c = [z|u|junk] for batch(p, c)
                zuT = wpool.tile([128, RW], BF16, tag="zuT", bufs=3)
                zt3 = zuT[:].rearrange("p (c e) -> p c e", e=128)
                nc.sync.dma_start(zt3, zu[:], transpose=True)
                zuT4 = zuT[:].rearrange("p (c e) -> p c e", e=128)
                z_v = zuT4[:, :, 0:H]
                u_v = zuT4[:, :, H : 2 * H]

                # epilogue, all into one f32 tile:
                # acc = sigmoid(z); acc = 1 - DT*acc; acc = hv*acc; acc += u
                acc = wpool.tile([128, FW], F32, tag="acc")
                acc3 = acc[:].rearrange("p (c h) -> p c h", h=H)
                nc.scalar.activation(acc3, z_v, SIG)
                nc.vector.tensor_scalar(
                    acc[:], acc[:], -DT, 1.0, mybir.AluOpType.mult,
                    mybir.AluOpType.add,
                )
                nc.vector.tensor_mul(acc[:], hv[:], acc[:])
                nc.vector.tensor_tensor(
                    acc3, acc3, u_v, op=mybir.AluOpType.add
                )
                nc.sync.dma_start(
                    out[t0 : t0 + MACRO2, :].rearrange("(p c) h -> p c h", c=N_SUB2),
                    acc3,
                )

            if bench:
                dbg_t = cpool.tile([128, 64], F32, name="dbg_t2", tag="dbg_t2")
                nc.gpsimd.memset(dbg_t[:], 0.0)
                nc.sync.dma_start(dbg[:], dbg_t[:])

    nc.compile()
    return nc


def _build_nc_v4(b_core, reps=1, bench=False, chunk=8192, trips=None,
                 mm_loop=None, pre_inject=False):
    """v4: v2's dataflow wrapped in a For_i hardware loop.

    This backend charges ~40-300us per UNIQUE instruction per run while
    extra For_i trips are nearly free (measured: 12 vs 72 trips of an
    8-matmul body cost the same).  v2 unrolled ~486 instructions; v4 has
    a ~30-instruction loop body executed b_core/chunk times via For_i
    with ds() dynamic DRAM offsets, so the per-run instruction tax drops
    ~16x and what remains is actual device work.
    """
    import concourse.bacc as bacc
    import concourse.mybir as mybir
    import concourse.tile as tile
    from concourse.bass import ds

    F32 = mybir.dt.float32
    BF16 = mybir.dt.bfloat16
    SIG = mybir.ActivationFunctionType.Sigmoid

    C = chunk
    n_sub = C // 128                  # transpose chunks per trip
    fw = n_sub * H                    # f32 working width per trip
    n_grp = C // 2048                 # psum eviction groups per trip

    nc = bacc.Bacc("TRN2", target_bir_lowering=False, debug=False)
    big = "Internal" if bench else None
    hid = nc.dram_tensor("hidden", [b_core, H], F32, kind=big or "ExternalInput")
    # pre_inject: xpad is batch-major x.T (chunk-independent layout, same
    # (p c) rearrange as hidden); legacy: feature-major permuted layout.
    xpad_shape = [b_core, 16] if pre_inject else [16, b_core]
    xpad = nc.dram_tensor("xpad", xpad_shape, BF16, kind=big or "ExternalInput")
    rhsw = nc.dram_tensor("rhsw", [64, 96], BF16, kind="ExternalInput")
    out = nc.dram_tensor("out", [b_core, H], F32, kind=big or "ExternalOutput")
    dbg = nc.dram_tensor("dbg", [128, 64], F32, kind="ExternalOutput") if bench else None

    with tile.TileContext(nc) as tc:
        with (
            tc.tile_pool(name="const", bufs=1) as cpool,
            tc.tile_pool(name="psum", bufs=1, space="PSUM") as ppool,
        ):
            w_sb = cpool.tile([64, 96], BF16)
            nc.sync.dma_start(w_sb[:], rhsw[:])

            hv = cpool.tile([128, fw], F32, name="hv4", tag="hv4")
            rbm = cpool.tile([128, C], BF16, name="rbm4", tag="rbm4")
            trans = cpool.tile([128, C], BF16, name="tr4", tag="tr4")
            zu = cpool.tile([128, C], BF16, name="zu4", tag="zu4")
            zuT = cpool.tile([128, C], BF16, name="zuT4", tag="zuT4")
            acc = cpool.tile([128, fw], F32, name="acc4", tag="acc4")
            ps_pair = [
                ppool.tile([96, 2048], F32, name=f"ps4_{i}", tag=f"ps4_{i}")
                for i in range(2)
            ]
            # pad regions read by the transposes but never written per-trip
            pad_lo = 64 if pre_inject else H
            pad = rbm[:].rearrange("p (c e) -> p c e", e=128)[:, :, pad_lo:128]
            nc.gpsimd.memset(pad, 0.0)
            nc.gpsimd.memset(zu[96:128, :], 0.0)

            hv3 = hv[:].rearrange("p (c h) -> p c h", h=H)
            rb3 = rbm[:].rearrange("p (c e) -> p c e", e=128)[:, :, 0:H]
            tr3 = trans[:].rearrange("p (c e) -> p c e", e=128)
            zt3 = zuT[:].rearrange("p (c e) -> p c e", e=128)
            acc3 = acc[:].rearrange("p (c h) -> p c h", h=H)

            loop_end = (trips * C) if trips is not None else b_core
            for _ in range(reps):
                with tc.For_i(0, loop_end, C) as t0:
                    nc.sync.dma_start(
                        hv3,
                        hid[ds(t0, C), :].rearrange("(p c) h -> p c h", c=n_sub),
                    )
                    if pre_inject:
                        # x rows into rbm pad cols 48:64 (parallel HWDGE
                        # queue, off the transpose->matmul serial chain)
                        nc.scalar.dma_start(
                            rbm[:].rearrange("p (c e) -> p c e", e=128)[:, :, 48:64],
                            xpad[ds(t0, C), :].rearrange(
                                "(p c) i -> p c i", c=n_sub),
                        )
                    nc.scalar.activation(rb3, hv3, SIG)
                    nc.sync.dma_start(tr3, rbm[:], transpose=True)
                    if not pre_inject:
                        nc.sync.dma_start(
                            trans[48:64, :].rearrange("p (c e) -> p c e", e=128),
                            xpad[:, ds(t0, C)].rearrange("i (c e) -> i c e", e=128),
                        )
                    if mm_loop is not None:
                        # nested hardware loop over mm_loop-column windows
                        # (uniques don't scale with C; inner iterations
                        # cost ~13.6us each, so wider windows = fewer
                        # barriers at +5 uniques per extra 2048)
                        with tc.For_i(0, C, mm_loop) as c0:
                            for s in range(mm_loop // 512):
                                psl = ps_pair[(512 * s) // 2048]
                                po = (512 * s) % 2048
                                nc.tensor.matmul(
                                    psl[:, po : po + 512],
                                    w_sb[:],
                                    trans[0:64, ds(c0, mm_loop)].rearrange(
                                        "p (s n) -> p s n", n=512
                                    )[:, s, :],
                                    start=True,
                                    stop=True,
                                )
                            for hh in range(mm_loop // 2048):
                                nc.vector.tensor_copy(
                                    zu[0:96, ds(c0, mm_loop)].rearrange(
                                        "r (hh n) -> r hh n", n=2048
                                    )[:, hh, :],
                                    ps_pair[hh][:],
                                )
                    else:
                        for g in range(n_grp):
                            ps = ps_pair[g % 2]
                            for s in range(4):
                                w = 4 * g + s
                                nc.tensor.matmul(
                                    ps[:, 512 * s : 512 * s + 512],
                                    w_sb[:],
                                    trans[0:64, 512 * w : 512 * w + 512],
                                    start=True,
                                    stop=True,
                                )
                            nc.vector.tensor_copy(
                                zu[0:96, 2048 * g : 2048 * g + 2048], ps[:]
                            )
                    nc.sync.dma_start(zt3, zu[:], transpose=True)
                    zuT4 = zuT[:].rearrange("p (c e) -> p c e", e=128)
                    nc.scalar.activation(acc3, zuT4[:, :, 0:H], SIG)
                    nc.vector.tensor_scalar(
                        acc[:], acc[:], -DT, 1.0, mybir.AluOpType.mult,
                        mybir.AluOpType.add,
                    )
                    nc.vector.tensor_mul(acc[:], hv[:], acc[:])
                    nc.vector.tensor_tensor(
                        acc3, acc3, zuT4[:, :, H : 2 * H], op=mybir.AluOpType.add
                    )
                    nc.sync.dma_start(
                        out[ds(t0, C), :].rearrange("(p c) h -> p c h", c=n_sub),
                        acc3,
                    )

            if bench:
                dbg_t = cpool.tile([128, 64], F32, name="dbg_t4", tag="dbg_t4")
                nc.gpsimd.memset(dbg_t[:], 0.0)
                nc.sync.dma_start(dbg[:], dbg_t[:])

    nc.compile()
    return nc


KCFG = {"chunk": 16384, "mm_loop": 2048, "pre_inject": True, "v5": False}


def _build_nc_v5(b_core, reps=1, bench=False, trips=None):
    """v5: two independent 8192-row streams per For_i trip.

    Per-trip serial chains (load->sigmoid->transpose->matmul->evict->
    transpose-back->epilogue->store) measured ~342us at chunk=16384;
    two dependence-free streams with separate tile sets and split HWDGE
    queues let the Tile scheduler overlap A's compute with B's DMA.
    x-inject is pre-transpose from a batch-major x.T (contiguous
    128-partition DMA), off the critical chain.
    """
    import concourse.bacc as bacc
    import concourse.mybir as mybir
    import concourse.tile as tile
    from concourse.bass import ds

    F32 = mybir.dt.float32
    BF16 = mybir.dt.bfloat16
    SIG = mybir.ActivationFunctionType.Sigmoid

    C2 = 8192                       # rows per stream
    n_sub = C2 // 128               # 64
    fw = n_sub * H                  # 3072
    STEP = 2 * C2

    nc = bacc.Bacc("TRN2", target_bir_lowering=False, debug=False)
    big = "Internal" if bench else None
    hid = nc.dram_tensor("hidden", [b_core, H], F32, kind=big or "ExternalInput")
    xpad = nc.dram_tensor("xpad", [b_core, 16], BF16, kind=big or "ExternalInput")
    rhsw = nc.dram_tensor("rhsw", [64, 96], BF16, kind="ExternalInput")
    out = nc.dram_tensor("out", [b_core, H], F32, kind=big or "ExternalOutput")
    dbg = nc.dram_tensor("dbg", [128, 64], F32, kind="ExternalOutput") if bench else None

    with tile.TileContext(nc) as tc:
        with (
            tc.tile_pool(name="const", bufs=1) as cpool,
            tc.tile_pool(name="psum", bufs=1, space="PSUM") as ppool,
        ):
            w_sb = cpool.tile([64, 96], BF16)
            nc.sync.dma_start(w_sb[:], rhsw[:])

            sets = []
            for sn in ("a", "b"):
                st = {
                    "hv": cpool.tile([128, fw], F32, name=f"hv5{sn}", tag=f"hv5{sn}"),
                    "rbm": cpool.tile([128, C2], BF16, name=f"rb5{sn}", tag=f"rb5{sn}"),
                    "trans": cpool.tile([128, C2], BF16, name=f"tr5{sn}", tag=f"tr5{sn}"),
                    "zu": cpool.tile([128, C2], BF16, name=f"zu5{sn}", tag=f"zu5{sn}"),
                    "zuT": cpool.tile([128, C2], BF16, name=f"zt5{sn}", tag=f"zt5{sn}"),
                    "acc": cpool.tile([128, fw], F32, name=f"ac5{sn}", tag=f"ac5{sn}"),
                    "ps": ppool.tile([96, 2048], F32, name=f"ps5{sn}", tag=f"ps5{sn}"),
                }
                pad = st["rbm"][:].rearrange("p (c e) -> p c e", e=128)[:, :, 64:128]
                nc.gpsimd.memset(pad, 0.0)
                nc.gpsimd.memset(st["zu"][96:128, :], 0.0)
                st["hv3"] = st["hv"][:].rearrange("p (c h) -> p c h", h=H)
                st["rb3"] = st["rbm"][:].rearrange("p (c e) -> p c e", e=128)[:, :, 0:H]
                st["rbx"] = st["rbm"][:].rearrange("p (c e) -> p c e", e=128)[:, :, 48:64]
                st["tr3"] = st["trans"][:].rearrange("p (c e) -> p c e", e=128)
                st["zt3"] = st["zuT"][:].rearrange("p (c e) -> p c e", e=128)
                st["acc3"] = st["acc"][:].rearrange("p (c h) -> p c h", h=H)
                sets.append(st)

            loop_end = (trips * STEP) if trips is not None else b_core
            for _ in range(reps):
                with tc.For_i(0, loop_end, STEP) as t0:
                    hid2 = hid[ds(t0, STEP), :].rearrange(
                        "(u p c) h -> u p c h", u=2, c=n_sub)
                    xp2 = xpad[ds(t0, STEP), :].rearrange(
                        "(u p c) i -> u p c i", u=2, c=n_sub)
                    out2 = out[ds(t0, STEP), :].rearrange(
                        "(u p c) h -> u p c h", u=2, c=n_sub)
                    qs = [nc.sync, nc.scalar]
                    for u, st in enumerate(sets):
                        qs[u % 2].dma_start(st["hv3"], hid2[u])
                        qs[(u + 1) % 2].dma_start(st["rbx"], xp2[u])
                        nc.scalar.activation(st["rb3"], st["hv3"], SIG)
                        qs[u % 2].dma_start(st["tr3"], st["rbm"][:], transpose=True)
                    with tc.For_i(0, C2, 2048) as c0:
                        for st in sets:
                            for s in range(4):
                                nc.tensor.matmul(
                                    st["ps"][:, 512 * s : 512 * s + 512],
                                    w_sb[:],
                                    st["trans"][0:64, ds(c0, 2048)].rearrange(
                                        "p (s n) -> p s n", n=512)[:, s, :],
                                    start=True,
                                    stop=True,
                                )
                            nc.vector.tensor_copy(
                                st["zu"][0:96, ds(c0, 2048)], st["ps"][:])
                    for u, st in enumerate(sets):
                        qs[u % 2].dma_start(st["zt3"], st["zu"][:], transpose=True)
                        zuT4 = st["zuT"][:].rearrange("p (c e) -> p c e", e=128)
                        nc.scalar.activation(st["acc3"], zuT4[:, :, 0:H], SIG)
                        nc.vector.tensor_scalar(
                            st["acc"][:], st["acc"][:], -DT, 1.0,
                            mybir.AluOpType.mult, mybir.AluOpType.add)
                        nc.vector.tensor_mul(st["acc"][:], st["hv"][:], st["acc"][:])
                        nc.vector.tensor_tensor(
                            st["acc3"], st["acc3"], zuT4[:, :, H : 2 * H],
                            op=mybir.AluOpType.add)
                        qs[u % 2].dma_start(out2[u], st["acc3"])

            if bench:
                dbg_t = cpool.tile([128, 64], F32, name="dbg_t5", tag="dbg_t5")
                nc.gpsimd.memset(dbg_t[:], 0.0)
                nc.sync.dma_start(dbg[:], dbg_t[:])

    nc.compile()
    return nc


def get_nc_v4(b_core=B_CORE, reps=1, bench=False, **over):
    cfg = {**KCFG, **over}
    use_v5 = cfg.pop("v5", False)
    key = ("v5" if use_v5 else "v4", b_core, reps, bench,
           tuple(sorted(cfg.items())))
    if key not in _NC_CACHE:
        if use_v5:
            _NC_CACHE[key] = _build_nc_v5(b_core, reps, bench)
        else:
            _NC_CACHE[key] = _build_nc_v4(b_core, reps, bench, **cfg)
    return _NC_CACHE[key]


def _build_xpad_v2(x, chunk=8192):
    """(16, B) = [x; ones; zeros] permuted for the v2/v4 inject.

    Device reads xpad[i, t0 + 128*c + e] as the x row for batch
    t0 + (chunk//128)*e + c (t0 = multiple of chunk).
    """
    n_sub = chunk // 128
    xz = np.zeros((16, x.shape[1]), np.float32)
    xz[0:IN] = x
    xz[IN] = 1.0
    v = xz.reshape(16, -1, 128, n_sub)                    # [i, m, e, c]
    w = np.ascontiguousarray(v.transpose(0, 1, 3, 2))     # [i, m, c, e]
    return w.reshape(16, x.shape[1])


def prepare_inputs_v2(hidden, x, P, b_v, K, C, P_z, b_z, e_e, e_i, n_cores=N_CORES,
                      chunk=8192, pre_inject=False):
    import ml_dtypes

    bf16 = ml_dtypes.bfloat16
    hidden = np.ascontiguousarray(np.asarray(hidden, np.float32))
    x = np.asarray(x, np.float32)
    rhs128 = _build_rhs(
        np.asarray(P), np.asarray(b_v), np.asarray(K), np.asarray(C),
        np.asarray(P_z), np.asarray(b_z), np.asarray(e_e), np.asarray(e_i),
    )
    blk = rhs128[0:64, 0:96].astype(bf16)                 # (64, 96)
    if pre_inject:
        # batch-major [B, 16] = [x; ones; zeros].T
        xz = np.zeros((16, x.shape[1]), np.float32)
        xz[0:IN] = x
        xz[IN] = 1.0
        xpad = np.ascontiguousarray(xz.T.astype(bf16))
    else:
        xpad = _build_xpad_v2(x, chunk).astype(bf16)
    b_core = hidden.shape[0] // n_cores
    in_maps = []
    for k in range(n_cores):
        s = slice(k * b_core, (k + 1) * b_core)
        in_maps.append(
            {
                "hidden": hidden[s],
                "xpad": np.ascontiguousarray(xpad[s] if pre_inject
                                             else xpad[:, s]),
                "rhsw": blk,
            }
        )
    return in_maps


def get_nc(b_core=B_CORE, reps=1, stage=5):
    key = (b_core, reps, stage)
    if key not in _NC_CACHE:
        _NC_CACHE[key] = _build_nc(b_core, reps, stage)
    return _NC_CACHE[key]


def prepare_inputs(hidden, x, P, b_v, K, C, P_z, b_z, e_e, e_i, n_cores=N_CORES):
    """Host-side prep: returns per-core in_maps."""
    import ml_dtypes

    bf16 = ml_dtypes.bfloat16
    hidden = np.ascontiguousarray(np.asarray(hidden, np.float32))
    x = np.asarray(x, np.float32)
    rhs = _build_rhs(
        np.asarray(P), np.asarray(b_v), np.asarray(K), np.asarray(C),
        np.asarray(P_z), np.asarray(b_z), np.asarray(e_e), np.asarray(e_i),
    ).astype(bf16)
    xpad = _build_xpad(x).astype(bf16)
    b_core = hidden.shape[0] // n_cores
    in_maps = []
    for k in range(n_cores):
        s = slice(k * b_core, (k + 1) * b_core)
        in_maps.append(
            {
                "hidden": hidden[s],
                "xpad": np.ascontiguousarray(xpad[:, s]),
                "rhsw": rhs,
            }
        )
    return in_maps


def get_nc_v2(b_core=B_CORE, reps=1, bench=False):
    key = ("v2", b_core, reps, bench)
    if key not in _NC_CACHE:
        _NC_CACHE[key] = _build_nc_v2(b_core, reps, bench)
    return _NC_CACHE[key]


def kernel(hidden, x, P, b_v, K, C, P_z, b_z, e_e, e_i):
    from concourse.bass_utils import run_bass_kernel_spmd

    nc = get_nc_v4(B_CORE)
    in_maps = prepare_inputs_v2(hidden, x, P, b_v, K, C, P_z, b_z, e_e, e_i,
                                chunk=KCFG["chunk"],
                                pre_inject=KCFG["pre_inject"] or KCFG["v5"])
    res = run_bass_kernel_spmd(nc, in_maps, list(range(N_CORES)))
    out = np.concatenate([r["out"] for r in res.results], axis=0)
    return out.astype(np.float32)



# revision 48
# speedup vs baseline: 1.6495x; 1.6495x over previous
"""Trainium2 Bass kernel for the Dale CB-cell step.

Math (per batch column b, H=48, IN=8):
    v      = hidden[b, :]                    (carried state)
    r      = sigmoid(v)
    zpre   = Ksp @ r + P_z @ x[:, b] + b_z
    u      = DT*(W @ r + P_masked @ x[:, b] + b_v)
    v_new  = v * (1 - DT*sigmoid(zpre)) + u

Sharding: pure batch data-parallel — each of the 8 cores gets 131072
rows of `hidden`/`x`; all weights are host-folded (softplus, Dale
masks, DT scaling, biases-via-ones-row) into one constant (64, 96)
bf16 matrix and replicated.

Active design (`_build_nc_v4`, used by `kernel()`): this backend
charges ~40-80us per UNIQUE instruction per run (program-size-driven;
superlinear), while For_i hardware-loop trip counts are nearly free
(measured: 12 vs 72 trips of an 8-matmul body cost the same wall).
v4 therefore wraps v2's proven dataflow in a For_i loop over
16384-row chunks with ds() dynamic DRAM offsets: per trip 1 hv load,
1 pre-transpose x-inject (batch-major x.T slab on the second HWDGE
queue, contiguous 128-partition pattern — ~0.9ms faster than the
legacy 16-partition scattered post-transpose inject), 1 ACT sigmoid,
1 DMA-xbar transpose to H-major, a nested For_i over 2048-column
windows (4 matmuls + 1 DVE psum eviction), 1 DMA-xbar transpose
back, a 4-op f32 epilogue and 1 store — ~25 unique instructions
total vs ~486 unrolled in v2, cutting the measured per-rep delta
from ~35.8ms to ~4.7ms (run-to-run drift 3.6-6.7ms; A/B only valid
back-to-back in one process). Inner-loop granularity 512 is WORSE
(inner iterations cost ~5-9us); 2048 is the sweet spot. Outer trips
cost ~342us each at chunk=16384; chunk=32768 does not fit SBUF. A
two-stream overlap variant (`_build_nc_v5`) LOST to v4 (extra unique
instructions outweigh overlap). `_build_nc` (v1) and `_build_nc_v2`
are kept for ablation only.
"""

import sys

if "/opt/trn_rl_repo" not in sys.path:
    sys.path.insert(0, "/opt/trn_rl_repo")

import numpy as np

H = 48
IN = 8
DT = 0.1
B = 1048576
N_CORES = 8
B_CORE = B // N_CORES          # 131072
MACRO = 2048                   # batch rows per macro-tile
N_SUB = MACRO // 128           # 16 subtiles per macro
N_CHUNK = N_SUB // 2           # 8 matmul chunks (2 subtiles each)
GQ = 256                       # psum column stride per chunk (bank-safe)

_NC_CACHE = {}


def _softplus64(x):
    x = x.astype(np.float64)
    return np.log1p(np.exp(-np.abs(x))) + np.maximum(x, 0.0)


def _build_rhs(P, b_v, K, C, P_z, b_z, e_e, e_i):
    """Host fold of all weights into the (128, 192) matmul rhs."""
    Ksp = _softplus64(K)
    Csp = _softplus64(C)
    S = Ksp + Csp
    e_e = float(np.asarray(e_e).reshape(-1)[0])
    e_i = float(np.asarray(e_i).reshape(-1)[0])
    W_E = np.maximum(e_e * S[:, : H // 2], 0.0)
    W_I = -np.maximum(-(e_i * S[:, H // 2 :]), 0.0)
    W = np.concatenate([W_E, W_I], axis=1)          # (H, H)
    rows = np.arange(H)
    keep = ~(((rows >= H // 4) & (rows < H // 2)) | (rows >= 3 * H // 4))
    P_masked = P.astype(np.float64) * keep[:, None]

    blk = np.zeros((64, 96), np.float64)
    blk[0:H, 0:H] = Ksp.T                     # z-half:  Ksp @ r
    blk[0:H, H : 2 * H] = (DT * W).T          # u-half:  DT * W @ r
    blk[H : H + IN, 0:H] = P_z.astype(np.float64).T
    blk[H : H + IN, H : 2 * H] = (DT * P_masked).T
    blk[H + IN, 0:H] = b_z.astype(np.float64).reshape(-1)
    blk[H + IN, H : 2 * H] = DT * b_v.astype(np.float64).reshape(-1)
    rhs = np.zeros((128, 192), np.float64)
    rhs[0:64, 0:96] = blk                     # even subtile rows
    rhs[64:128, 96:192] = blk                 # odd subtile rows
    return rhs


def _build_xpad(x):
    """(16, B) = [x; ones; zeros] permuted to the device batch layout.

    Device reads xpad[i, t0 + 256*cc + 128*a + e] as the x row for batch
    index t0 + 16*e + 2*cc + a.
    """
    xz = np.zeros((16, x.shape[1]), np.float32)
    xz[0:IN] = x
    xz[IN] = 1.0
    v = xz.reshape(16, -1, 128, 16)           # [i, m, e, s]
    v = v.reshape(16, v.shape[1], 128, 8, 2)  # [i, m, e, cc, a]
    w = np.ascontiguousarray(v.transpose(0, 1, 3, 4, 2))  # [i, m, cc, a, e]
    return w.reshape(16, x.shape[1])


def _build_nc(b_core, reps=1, stage=5, bench=False):
    """reps>1 repeats the whole body in one NEFF (for delta-timing).

    stage: ablation ladder for bottleneck isolation (5 = full kernel):
      0 DMA only (hv load + x injects + store hv)
      1 + ACT r-sigmoid + DMA-transpose (consumed via tiny scratch store)
      2 + matmuls (psum slice consumed via tiny scratch store)
      3 + ACT z-sigmoid (zs slice consumed via tiny scratch store)
      4 + DVE g/v_term (vt stored as output)
      5 full
    """
    import concourse.bacc as bacc
    import concourse.mybir as mybir
    import concourse.tile as tile

    F32 = mybir.dt.float32
    BF16 = mybir.dt.bfloat16
    SIG = mybir.ActivationFunctionType.Sigmoid

    n_macro = b_core // MACRO
    nc = bacc.Bacc("TRN2", target_bir_lowering=False, debug=False)
    # bench mode: big tensors are device-internal (uninitialized) so runs
    # carry no host<->device transfer; timing-only, results meaningless.
    big = "Internal" if bench else None
    hid = nc.dram_tensor("hidden", [b_core, H], F32, kind=big or "ExternalInput")
    xpad = nc.dram_tensor("xpad", [16, b_core], BF16, kind=big or "ExternalInput")
    rhsw = nc.dram_tensor("rhsw", [128, 192], BF16, kind="ExternalInput")
    out = nc.dram_tensor("out", [b_core, H], F32, kind=big or "ExternalOutput")
    dbg = nc.dram_tensor("dbg", [128, 64], F32, kind="ExternalOutput") if bench else None
    scratch = (
        nc.dram_tensor("scratch", [128, 256], F32) if stage in (2, 3) else None
    )
    scratchb = (
        nc.dram_tensor("scratchb", [128, 64], BF16) if stage in (0, 1) else None
    )

    FW = N_SUB * H                            # 768  f32 working width
    RW = N_SUB * 64                           # 1024 bf16 padded width

    with tile.TileContext(nc) as tc:
        with (
            tc.tile_pool(name="const", bufs=1) as cpool,
            tc.tile_pool(name="io", bufs=3) as iopool,
            tc.tile_pool(name="work", bufs=2) as wpool,
            tc.tile_pool(name="psum", bufs=2, space="PSUM") as ppool,
        ):
            rhs_sb = cpool.tile([128, 192], BF16)
            nc.sync.dma_start(rhs_sb[:], rhsw[:])

            # r staging buffers are manually double-buffered so their pad
            # columns can be zeroed exactly once (pool slot rotation would
            # leave junk/NaN bits there for the DMA transpose to read).
            rbm_bufs = [
                cpool.tile([128, RW], BF16, name=f"rbm{i}", tag=f"rbm{i}")
                for i in range(2)
            ]
            for rb in rbm_bufs:
                pad = rb[:].rearrange("p (c e) -> p c e", e=64)[:, :, H:64]
                nc.gpsimd.memset(pad, 0.0)

            for m in range(n_macro * reps):
                t0 = (m % n_macro) * MACRO

                hv = iopool.tile([128, FW], F32, tag="hv")
                hv3 = hv[:].rearrange("p (c h) -> p c h", h=H)
                nc.sync.dma_start(
                    hv3, hid[t0 : t0 + MACRO, :].rearrange("(p c) h -> p c h", c=N_SUB)
                )

                lhsT = wpool.tile([128, RW], BF16, tag="lhsT")
                if stage >= 1:
                    # r = sigmoid(v), bf16, in 64-col padded blocks
                    rbm = rbm_bufs[m % 2]
                    rb3 = rbm[:].rearrange("p (c e) -> p c e", e=64)[:, :, 0:H]
                    nc.scalar.activation(rb3, hv3, SIG)

                    # H-major activations: chunk cc of lhsT = transpose of
                    # rbm cols [128cc, 128cc+128)
                    lt3 = lhsT[:].rearrange("p (c e) -> p c e", e=128)
                    nc.sync.dma_start(lt3, rbm[:], transpose=True)

                # x/ones/zeros into the pad partitions
                xsrc = xpad[:, t0 : t0 + MACRO].rearrange("i (c e) -> i c e", e=256)
                nc.sync.dma_start(
                    lhsT[48:64, :].rearrange("p (c e) -> p c e", e=128),
                    xsrc[:, :, 0:128],
                )
                nc.sync.dma_start(
                    lhsT[112:128, :].rearrange("p (c e) -> p c e", e=128),
                    xsrc[:, :, 128:256],
                )
                if stage <= 1:
                    nc.sync.dma_start(scratchb[:, 0:64], lhsT[:, 0:64])
                    nc.sync.dma_start(
                        out[t0 : t0 + MACRO, :].rearrange("(p c) h -> p c h", c=N_SUB),
                        hv3,
                    )
                    continue

                ps = ppool.tile([128, N_CHUNK * GQ], F32, tag="ps")
                for cc in range(N_CHUNK):
                    nc.tensor.matmul(
                        ps[:, GQ * cc : GQ * cc + 192],
                        lhsT[:, 128 * cc : 128 * cc + 128],
                        rhs_sb[:],
                        start=True,
                        stop=True,
                    )
                if stage == 2:
                    tmp = wpool.tile([128, 64], F32, tag="pscopy")
                    nc.scalar.activation(
                        tmp[:], ps[:, 0:64], mybir.ActivationFunctionType.Copy
                    )
                    nc.sync.dma_start(scratch[:, 0:64], tmp[:])
                    nc.sync.dma_start(
                        out[t0 : t0 + MACRO, :].rearrange("(p c) h -> p c h", c=N_SUB),
                        hv3,
                    )
                    continue

                ps4 = (
                    ps[:]
                    .rearrange("p (g q) -> p g q", q=GQ)[:, :, 0:192]
                    .rearrange("p g (a x) -> p g a x", x=96)
                )
                ps_z = ps4[:, :, :, 0:H]
                ps_u = ps4[:, :, :, H : 2 * H]

                zs = wpool.tile([128, FW], F32, tag="zs")
                zs4 = zs[:].rearrange("p (g a x) -> p g a x", g=N_CHUNK, a=2)
                nc.scalar.activation(zs4, ps_z, SIG)
                if stage == 3:
                    nc.sync.dma_start(scratch[:, 0:64], zs[:, 0:64])
                    nc.sync.dma_start(
                        out[t0 : t0 + MACRO, :].rearrange("(p c) h -> p c h", c=N_SUB),
                        hv3,
                    )
                    continue

                gt = wpool.tile([128, FW], F32, tag="gt")
                nc.vector.tensor_scalar(
                    gt[:], zs[:], -DT, 1.0, mybir.AluOpType.mult, mybir.AluOpType.add
                )

                vt = wpool.tile([128, FW], F32, tag="vt")
                nc.vector.tensor_mul(vt[:], hv[:], gt[:])
                if stage == 4:
                    nc.sync.dma_start(
                        out[t0 : t0 + MACRO, :].rearrange("(p c) h -> p c h", c=N_SUB),
                        vt[:].rearrange("p (c h) -> p c h", h=H),
                    )
                    continue

                ot = iopool.tile([128, FW], F32, tag="ot")
                ot4 = ot[:].rearrange("p (g a x) -> p g a x", g=N_CHUNK, a=2)
                vt4 = vt[:].rearrange("p (g a x) -> p g a x", g=N_CHUNK, a=2)
                nc.vector.tensor_add(ot4, vt4, ps_u)

                nc.sync.dma_start(
                    out[t0 : t0 + MACRO, :].rearrange("(p c) h -> p c h", c=N_SUB),
                    ot[:].rearrange("p (c h) -> p c h", h=H),
                )

            if bench:
                dbg_t = cpool.tile([128, 64], F32, name="dbg_t", tag="dbg_t")
                nc.gpsimd.memset(dbg_t[:], 0.0)
                nc.sync.dma_start(dbg[:], dbg_t[:])

    nc.compile()
    return nc


MACRO2 = 8192                 # v2 macro-tile rows
N_SUB2 = MACRO2 // 128        # 64 subtiles (one per 128-col transpose chunk)
N_WIN = MACRO2 // 512         # 16 matmul windows per macro


def _build_nc_v2(b_core, reps=1, bench=False, mm_n=512):
    """v2: instruction-count-minimized design.

    Per 8192-row macro: 1 hv load, 1 sigmoid, 1 DMA-transpose (1 subtile
    per 128-chunk, K rows 0:64 = [r(48); x/ones/zeros(16)]), 1 x-inject,
    16 matmuls (lhsT = constant (64, 96) weights, rhs = 512 batch cols),
    4 DVE psum evictions to bf16, 1 DMA-transpose back to batch-major,
    then a 4-op in-place f32 epilogue + 1 store.
    """
    import concourse.bacc as bacc
    import concourse.mybir as mybir
    import concourse.tile as tile

    F32 = mybir.dt.float32
    BF16 = mybir.dt.bfloat16
    SIG = mybir.ActivationFunctionType.Sigmoid

    n_macro = b_core // MACRO2
    nc = bacc.Bacc("TRN2", target_bir_lowering=False, debug=False)
    big = "Internal" if bench else None
    hid = nc.dram_tensor("hidden", [b_core, H], F32, kind=big or "ExternalInput")
    xpad = nc.dram_tensor("xpad", [16, b_core], BF16, kind=big or "ExternalInput")
    rhsw = nc.dram_tensor("rhsw", [64, 96], BF16, kind="ExternalInput")
    out = nc.dram_tensor("out", [b_core, H], F32, kind=big or "ExternalOutput")
    dbg = nc.dram_tensor("dbg", [128, 64], F32, kind="ExternalOutput") if bench else None

    FW = N_SUB2 * H               # 3072  (f32 working width per macro)
    RW = N_SUB2 * 128             # 8192  (bf16 padded width per macro)

    with tile.TileContext(nc) as tc:
        with (
            tc.tile_pool(name="const", bufs=1) as cpool,
            tc.tile_pool(name="io", bufs=2) as iopool,
            tc.tile_pool(name="work", bufs=2) as wpool,
            tc.tile_pool(name="psum", bufs=2, space="PSUM") as ppool,
        ):
            w_sb = cpool.tile([64, 96], BF16)
            nc.sync.dma_start(w_sb[:], rhsw[:])

            # manual double-buffers for the two transpose sources so their
            # never-written pad regions can be zeroed exactly once (keeps
            # CoreSim's uninit-read check green; HW wouldn't care).
            rbm_bufs = [
                cpool.tile([128, RW], BF16, name=f"rbm2_{i}", tag=f"rbm2_{i}")
                for i in range(2)
            ]
            for rb in rbm_bufs:
                pad = rb[:].rearrange("p (c e) -> p c e", e=128)[:, :, H:128]
                nc.gpsimd.memset(pad, 0.0)
            zu_bufs = [
                cpool.tile([128, RW], BF16, name=f"zu2_{i}", tag=f"zu2_{i}")
                for i in range(2)
            ]
            for zb in zu_bufs:
                nc.gpsimd.memset(zb[96:128, :], 0.0)

            for m in range(n_macro * reps):
                t0 = (m % n_macro) * MACRO2

                # batch(p, c) = t0 + 64*p + c, c in [0, 64)
                hv = iopool.tile([128, FW], F32, tag="hv", bufs=3)
                hv3 = hv[:].rearrange("p (c h) -> p c h", h=H)
                nc.sync.dma_start(
                    hv3,
                    hid[t0 : t0 + MACRO2, :].rearrange("(p c) h -> p c h", c=N_SUB2),
                )

                # r = sigmoid(v) bf16 into 128-col padded chunks
                rbm = rbm_bufs[m % 2]
                rb3 = rbm[:].rearrange("p (c e) -> p c e", e=128)[:, :, 0:H]
                nc.scalar.activation(rb3, hv3, SIG)

                # chunk c of trans = transpose of rbm cols [128c, 128c+128):
                # rows 0:48 = r (H-major), 48:64 <- x/ones/zeros, 64:128 junk
                trans = wpool.tile([128, RW], BF16, tag="trans")
                tr3 = trans[:].rearrange("p (c e) -> p c e", e=128)
                nc.sync.dma_start(tr3, rbm[:], transpose=True)
                nc.sync.dma_start(
                    trans[48:64, :].rearrange("p (c e) -> p c e", e=128),
                    xpad[:, t0 : t0 + MACRO2].rearrange("i (c e) -> i c e", e=128),
                )

                # zu: cols 128c+e <-> batch(e, c); rows [z(48) | u(48)]
                zu = zu_bufs[m % 2]
                mm_per_ps = 2048 // mm_n
                for g in range(4):
                    ps = ppool.tile([96, 2048], F32, tag="ps")
                    for s in range(mm_per_ps):
                        w = mm_per_ps * g + s
                        nc.tensor.matmul(
                            ps[:, mm_n * s : mm_n * s + mm_n],
                            w_sb[:],
                            trans[0:64, mm_n * w : mm_n * w + mm_n],
                            start=True,
                            stop=True,
                        )
                    nc.vector.tensor_copy(
                        zu[0:96, 2048 * g : 2048 * g + 2048], ps[:]
                    )

                # back to batch-major: zuT chunk c = [z|u|junk] for batch(p, c)
                zuT = wpool.tile([128, RW], BF16, tag="zuT", bufs=3)
                zt3 = zuT[:].rearrange("p (c e) -> p c e", e=128)
                nc.sync.dma_start(zt3, zu[:], transpose=True)
                zuT4 = zuT[:].rearrange("p (c e) -> p c e", e=128)
                z_v = zuT4[:, :, 0:H]
                u_v = zuT4[:, :, H : 2 * H]

                # epilogue, all into one f32 tile:
                # acc = sigmoid(z); acc = 1 - DT*acc; acc = hv*acc; acc += u
                acc = wpool.tile([128, FW], F32, tag="acc")
                acc3 = acc[:].rearrange("p (c h) -> p c h", h=H)
                nc.scalar.activation(acc3, z_v, SIG)
                nc.vector.tensor_scalar(
                    acc[:], acc[:], -DT, 1.0, mybir.AluOpType.mult,
                    mybir.AluOpType.add,
                )
                nc.vector.tensor_mul(acc[:], hv[:], acc[:])
                nc.vector.tensor_tensor(
                    acc3, acc3, u_v, op=mybir.AluOpType.add
                )
                nc.sync.dma_start(
                    out[t0 : t0 + MACRO2, :].rearrange("(p c) h -> p c h", c=N_SUB2),
                    acc3,
                )

            if bench:
                dbg_t = cpool.tile([128, 64], F32, name="dbg_t2", tag="dbg_t2")
                nc.gpsimd.memset(dbg_t[:], 0.0)
                nc.sync.dma_start(dbg[:], dbg_t[:])

    nc.compile()
    return nc


def _build_nc_v4(b_core, reps=1, bench=False, chunk=8192, trips=None,
                 mm_loop=None, pre_inject=False, pack64=False,
                 evict_split=False):
    """v4: v2's dataflow wrapped in a For_i hardware loop.

    This backend charges ~40-300us per UNIQUE instruction per run while
    extra For_i trips are nearly free (measured: 12 vs 72 trips of an
    8-matmul body cost the same).  v2 unrolled ~486 instructions; v4 has
    a ~30-instruction loop body executed b_core/chunk times via For_i
    with ds() dynamic DRAM offsets, so the per-run instruction tax drops
    ~16x and what remains is actual device work.
    """
    import concourse.bacc as bacc
    import concourse.mybir as mybir
    import concourse.tile as tile
    from concourse.bass import ds

    F32 = mybir.dt.float32
    BF16 = mybir.dt.bfloat16
    SIG = mybir.ActivationFunctionType.Sigmoid

    C = chunk
    n_sub = C // 128                  # transpose chunks per trip
    fw = n_sub * H                    # f32 working width per trip
    n_grp = C // 2048                 # psum eviction groups per trip
    assert not pack64 or pre_inject, "pack64 requires pre_inject"
    assert not evict_split or (mm_loop and not pack64), (
        "evict_split is only implemented in the mm_loop branch"
    )

    nc = bacc.Bacc("TRN2", target_bir_lowering=False, debug=False)
    big = "Internal" if bench else None
    hid = nc.dram_tensor("hidden", [b_core, H], F32, kind=big or "ExternalInput")
    # pre_inject: xpad is batch-major x.T (chunk-independent layout, same
    # (p c) rearrange as hidden); legacy: feature-major permuted layout.
    xpad_shape = [b_core, 16] if pre_inject else [16, b_core]
    xpad = nc.dram_tensor("xpad", xpad_shape, BF16, kind=big or "ExternalInput")
    # evict_split pads the weight columns so z lands at psum rows 0:48
    # and u at rows 64:112 — engine PSUM access must start on a
    # 32-partition strip boundary (offset 48 fails BIR verification).
    rw = 128 if evict_split else 96
    rhsw = nc.dram_tensor("rhsw", [64, rw], BF16, kind="ExternalInput")
    out = nc.dram_tensor("out", [b_core, H], F32, kind=big or "ExternalOutput")
    dbg = nc.dram_tensor("dbg", [128, 64], F32, kind="ExternalOutput") if bench else None

    with tile.TileContext(nc) as tc:
        with (
            tc.tile_pool(name="const", bufs=1) as cpool,
            tc.tile_pool(name="psum", bufs=1, space="PSUM") as ppool,
        ):
            w_sb = cpool.tile([64, rw], BF16)
            nc.sync.dma_start(w_sb[:], rhsw[:])

            # pack64: rbm chunks are 64-wide [r48|x16] (no 128-pad), so the
            # forward transpose moves half the bytes and the transposed
            # tile is half as wide (half the inner-loop iterations); each
            # transposed 128-chunk stacks two subtiles (rows 0:64 / 64:128)
            # handled by separate A/B matmuls into separate psum tiles.
            rbw = C // 2 if pack64 else C
            ew = 64 if pack64 else 128
            hv = cpool.tile([128, fw], F32, name="hv4", tag="hv4")
            rbm = cpool.tile([128, rbw], BF16, name="rbm4", tag="rbm4")
            trans = cpool.tile([128, rbw], BF16, name="tr4", tag="tr4")
            zu = cpool.tile([128, C], BF16, name="zu4", tag="zu4")
            zuT = cpool.tile([128, C], BF16, name="zuT4", tag="zuT4")
            acc = cpool.tile([128, fw], F32, name="acc4", tag="acc4")
            ps_pair = [
                ppool.tile([rw, 2048], F32, name=f"ps4_{i}", tag=f"ps4_{i}")
                for i in range(2)
            ]
            # pad regions read by the transposes but never written per-trip
            if not (pack64 and pre_inject):
                pad_lo = 64 if pre_inject else H
                pad = rbm[:].rearrange("p (c e) -> p c e", e=ew)[:, :, pad_lo:ew]
                nc.gpsimd.memset(pad, 0.0)
            nc.gpsimd.memset(zu[96:128, :], 0.0)
            if evict_split:
                # rows 48:64 sit between the z and u eviction bands and
                # are never written per-trip; engine SBUF/PSUM access
                # must start on a 32-partition boundary, so zero 32:64
                # (32:48 is harmlessly re-written by the z-evict)
                nc.gpsimd.memset(zu[32:64, :], 0.0)

            hv3 = hv[:].rearrange("p (c h) -> p c h", h=H)
            rb3 = rbm[:].rearrange("p (c e) -> p c e", e=ew)[:, :, 0:H]
            tr3 = trans[:].rearrange("p (c e) -> p c e", e=128)
            zt3 = zuT[:].rearrange("p (c e) -> p c e", e=128)
            acc3 = acc[:].rearrange("p (c h) -> p c h", h=H)

            loop_end = (trips * C) if trips is not None else b_core
            for _ in range(reps):
                with tc.For_i(0, loop_end, C) as t0:
                    nc.sync.dma_start(
                        hv3,
                        hid[ds(t0, C), :].rearrange("(p c) h -> p c h", c=n_sub),
                    )
                    if pre_inject:
                        # x rows into rbm pad cols 48:64 (parallel HWDGE
                        # queue, off the transpose->matmul serial chain)
                        nc.scalar.dma_start(
                            rbm[:].rearrange("p (c e) -> p c e", e=ew)[:, :, 48:64],
                            xpad[ds(t0, C), :].rearrange(
                                "(p c) i -> p c i", c=n_sub),
                        )
                    nc.scalar.activation(rb3, hv3, SIG)
                    nc.sync.dma_start(tr3, rbm[:], transpose=True)
                    if not pre_inject:
                        nc.sync.dma_start(
                            trans[48:64, :].rearrange("p (c e) -> p c e", e=128),
                            xpad[:, ds(t0, C)].rearrange("i (c e) -> i c e", e=128),
                        )
                    if pack64:
                        # trans chunk c' stacks subtiles 2c' (rows 0:64)
                        # and 2c'+1 (rows 64:128); per group of 2048 trans
                        # cols: 4 A-mms + 4 B-mms, evicted to interleaved
                        # 128-col blocks of zu (even/odd subtiles).
                        n_g = rbw // 2048
                        zu5 = zu[0:96].rearrange(
                            "r (g c2 two e) -> r g c2 two e",
                            g=n_g, two=2, e=128)
                        trg = [
                            trans[64 * u : 64 * u + 64].rearrange(
                                "p (g n) -> p g n", n=2048)
                            for u in range(2)
                        ]
                        with tc.For_i(0, n_g, 1) as g0:
                            for u in range(2):
                                for s in range(4):
                                    nc.tensor.matmul(
                                        ps_pair[u][:, 512 * s : 512 * s + 512],
                                        w_sb[:],
                                        trg[u][:, ds(g0, 1), :][
                                            :, 0, 512 * s : 512 * s + 512],
                                        start=True,
                                        stop=True,
                                    )
                            for u in range(2):
                                nc.vector.tensor_copy(
                                    zu5[:, ds(g0, 1), :, u, :][:, 0],
                                    ps_pair[u][:],
                                )
                    elif mm_loop is not None:
                        # nested hardware loop over mm_loop-column windows
                        # (uniques don't scale with C; inner iterations
                        # cost ~13.6us each, so wider windows = fewer
                        # barriers at +5 uniques per extra 2048)
                        with tc.For_i(0, C, mm_loop) as c0:
                            for s in range(mm_loop // 512):
                                psl = ps_pair[(512 * s) // 2048]
                                po = (512 * s) % 2048
                                nc.tensor.matmul(
                                    psl[:, po : po + 512],
                                    w_sb[:],
                                    trans[0:64, ds(c0, mm_loop)].rearrange(
                                        "p (s n) -> p s n", n=512
                                    )[:, s, :],
                                    start=True,
                                    stop=True,
                                )
                            for hh in range(mm_loop // 2048):
                                if evict_split:
                                    # sigma fused into the z-half evict
                                    # (ACT reads psum rows 0:48); u-half
                                    # (rows 64:128, strip-aligned) via
                                    # DVE. Kills the post-transpose
                                    # sigmoid + one chain hop; z is bf16-
                                    # quantized AFTER sigma (better).
                                    nc.scalar.activation(
                                        zu[0:48, ds(c0, mm_loop)].rearrange(
                                            "r (hh n) -> r hh n", n=2048
                                        )[:, hh, :],
                                        ps_pair[hh][0:48, :],
                                        SIG,
                                    )
                                    nc.vector.tensor_copy(
                                        zu[64:128, ds(c0, mm_loop)].rearrange(
                                            "r (hh n) -> r hh n", n=2048
                                        )[:, hh, :],
                                        ps_pair[hh][64:128, :],
                                    )
                                else:
                                    nc.vector.tensor_copy(
                                        zu[0:96, ds(c0, mm_loop)].rearrange(
                                            "r (hh n) -> r hh n", n=2048
                                        )[:, hh, :],
                                        ps_pair[hh][:],
                                    )
                    else:
                        for g in range(n_grp):
                            ps = ps_pair[g % 2]
                            for s in range(4):
                                w = 4 * g + s
                                nc.tensor.matmul(
                                    ps[:, 512 * s : 512 * s + 512],
                                    w_sb[:],
                                    trans[0:64, 512 * w : 512 * w + 512],
                                    start=True,
                                    stop=True,
                                )
                            nc.vector.tensor_copy(
                                zu[0:96, 2048 * g : 2048 * g + 2048], ps[:]
                            )
                    nc.sync.dma_start(zt3, zu[:], transpose=True)
                    zuT4 = zuT[:].rearrange("p (c e) -> p c e", e=128)
                    if evict_split:
                        # zuT z-half already holds sigma(zpre)
                        nc.vector.tensor_scalar(
                            acc3, zuT4[:, :, 0:H], -DT, 1.0,
                            mybir.AluOpType.mult, mybir.AluOpType.add,
                        )
                    else:
                        nc.scalar.activation(acc3, zuT4[:, :, 0:H], SIG)
                        nc.vector.tensor_scalar(
                            acc[:], acc[:], -DT, 1.0, mybir.AluOpType.mult,
                            mybir.AluOpType.add,
                        )
                    nc.vector.tensor_mul(acc[:], hv[:], acc[:])
                    u_lo = 64 if evict_split else H
                    nc.vector.tensor_tensor(
                        acc3, acc3, zuT4[:, :, u_lo : u_lo + H],
                        op=mybir.AluOpType.add
                    )
                    nc.sync.dma_start(
                        out[ds(t0, C), :].rearrange("(p c) h -> p c h", c=n_sub),
                        acc3,
                    )

            if bench:
                dbg_t = cpool.tile([128, 64], F32, name="dbg_t4", tag="dbg_t4")
                nc.gpsimd.memset(dbg_t[:], 0.0)
                nc.sync.dma_start(dbg[:], dbg_t[:])

    nc.compile()
    return nc


KCFG = {"chunk": 16384, "mm_loop": 2048, "pre_inject": True, "v5": False,
        "pack64": False, "evict_split": False}


def _build_nc_v5(b_core, reps=1, bench=False, trips=None):
    """v5: two independent 8192-row streams per For_i trip.

    Per-trip serial chains (load->sigmoid->transpose->matmul->evict->
    transpose-back->epilogue->store) measured ~342us at chunk=16384;
    two dependence-free streams with separate tile sets and split HWDGE
    queues let the Tile scheduler overlap A's compute with B's DMA.
    x-inject is pre-transpose from a batch-major x.T (contiguous
    128-partition DMA), off the critical chain.
    """
    import concourse.bacc as bacc
    import concourse.mybir as mybir
    import concourse.tile as tile
    from concourse.bass import ds

    F32 = mybir.dt.float32
    BF16 = mybir.dt.bfloat16
    SIG = mybir.ActivationFunctionType.Sigmoid

    C2 = 8192                       # rows per stream
    n_sub = C2 // 128               # 64
    fw = n_sub * H                  # 3072
    STEP = 2 * C2

    nc = bacc.Bacc("TRN2", target_bir_lowering=False, debug=False)
    big = "Internal" if bench else None
    hid = nc.dram_tensor("hidden", [b_core, H], F32, kind=big or "ExternalInput")
    xpad = nc.dram_tensor("xpad", [b_core, 16], BF16, kind=big or "ExternalInput")
    rhsw = nc.dram_tensor("rhsw", [64, 96], BF16, kind="ExternalInput")
    out = nc.dram_tensor("out", [b_core, H], F32, kind=big or "ExternalOutput")
    dbg = nc.dram_tensor("dbg", [128, 64], F32, kind="ExternalOutput") if bench else None

    with tile.TileContext(nc) as tc:
        with (
            tc.tile_pool(name="const", bufs=1) as cpool,
            tc.tile_pool(name="psum", bufs=1, space="PSUM") as ppool,
        ):
            w_sb = cpool.tile([64, 96], BF16)
            nc.sync.dma_start(w_sb[:], rhsw[:])

            sets = []
            for sn in ("a", "b"):
                st = {
                    "hv": cpool.tile([128, fw], F32, name=f"hv5{sn}", tag=f"hv5{sn}"),
                    "rbm": cpool.tile([128, C2], BF16, name=f"rb5{sn}", tag=f"rb5{sn}"),
                    "trans": cpool.tile([128, C2], BF16, name=f"tr5{sn}", tag=f"tr5{sn}"),
                    "zu": cpool.tile([128, C2], BF16, name=f"zu5{sn}", tag=f"zu5{sn}"),
                    "zuT": cpool.tile([128, C2], BF16, name=f"zt5{sn}", tag=f"zt5{sn}"),
                    "acc": cpool.tile([128, fw], F32, name=f"ac5{sn}", tag=f"ac5{sn}"),
                    "ps": ppool.tile([96, 2048], F32, name=f"ps5{sn}", tag=f"ps5{sn}"),
                }
                pad = st["rbm"][:].rearrange("p (c e) -> p c e", e=128)[:, :, 64:128]
                nc.gpsimd.memset(pad, 0.0)
                nc.gpsimd.memset(st["zu"][96:128, :], 0.0)
                st["hv3"] = st["hv"][:].rearrange("p (c h) -> p c h", h=H)
                st["rb3"] = st["rbm"][:].rearrange("p (c e) -> p c e", e=128)[:, :, 0:H]
                st["rbx"] = st["rbm"][:].rearrange("p (c e) -> p c e", e=128)[:, :, 48:64]
                st["tr3"] = st["trans"][:].rearrange("p (c e) -> p c e", e=128)
                st["zt3"] = st["zuT"][:].rearrange("p (c e) -> p c e", e=128)
                st["acc3"] = st["acc"][:].rearrange("p (c h) -> p c h", h=H)
                sets.append(st)

            loop_end = (trips * STEP) if trips is not None else b_core
            for _ in range(reps):
                with tc.For_i(0, loop_end, STEP) as t0:
                    hid2 = hid[ds(t0, STEP), :].rearrange(
                        "(u p c) h -> u p c h", u=2, c=n_sub)
                    xp2 = xpad[ds(t0, STEP), :].rearrange(
                        "(u p c) i -> u p c i", u=2, c=n_sub)
                    out2 = out[ds(t0, STEP), :].rearrange(
                        "(u p c) h -> u p c h", u=2, c=n_sub)
                    qs = [nc.sync, nc.scalar]
                    for u, st in enumerate(sets):
                        qs[u % 2].dma_start(st["hv3"], hid2[u])
                        qs[(u + 1) % 2].dma_start(st["rbx"], xp2[u])
                        nc.scalar.activation(st["rb3"], st["hv3"], SIG)
                        qs[u % 2].dma_start(st["tr3"], st["rbm"][:], transpose=True)
                    with tc.For_i(0, C2, 2048) as c0:
                        for st in sets:
                            for s in range(4):
                                nc.tensor.matmul(
                                    st["ps"][:, 512 * s : 512 * s + 512],
                                    w_sb[:],
                                    st["trans"][0:64, ds(c0, 2048)].rearrange(
                                        "p (s n) -> p s n", n=512)[:, s, :],
                                    start=True,
                                    stop=True,
                                )
                            nc.vector.tensor_copy(
                                st["zu"][0:96, ds(c0, 2048)], st["ps"][:])
                    for u, st in enumerate(sets):
                        qs[u % 2].dma_start(st["zt3"], st["zu"][:], transpose=True)
                        zuT4 = st["zuT"][:].rearrange("p (c e) -> p c e", e=128)
                        nc.scalar.activation(st["acc3"], zuT4[:, :, 0:H], SIG)
                        nc.vector.tensor_scalar(
                            st["acc"][:], st["acc"][:], -DT, 1.0,
                            mybir.AluOpType.mult, mybir.AluOpType.add)
                        nc.vector.tensor_mul(st["acc"][:], st["hv"][:], st["acc"][:])
                        nc.vector.tensor_tensor(
                            st["acc3"], st["acc3"], zuT4[:, :, H : 2 * H],
                            op=mybir.AluOpType.add)
                        qs[u % 2].dma_start(out2[u], st["acc3"])

            if bench:
                dbg_t = cpool.tile([128, 64], F32, name="dbg_t5", tag="dbg_t5")
                nc.gpsimd.memset(dbg_t[:], 0.0)
                nc.sync.dma_start(dbg[:], dbg_t[:])

    nc.compile()
    return nc


def get_nc_v4(b_core=B_CORE, reps=1, bench=False, **over):
    cfg = {**KCFG, **over}
    use_v5 = cfg.pop("v5", False)
    key = ("v5" if use_v5 else "v4", b_core, reps, bench,
           tuple(sorted(cfg.items())))
    if key not in _NC_CACHE:
        if use_v5:
            _NC_CACHE[key] = _build_nc_v5(b_core, reps, bench)
        else:
            _NC_CACHE[key] = _build_nc_v4(b_core, reps, bench, **cfg)
    return _NC_CACHE[key]


def _build_xpad_v2(x, chunk=8192):
    """(16, B) = [x; ones; zeros] permuted for the v2/v4 inject.

    Device reads xpad[i, t0 + 128*c + e] as the x row for batch
    t0 + (chunk//128)*e + c (t0 = multiple of chunk).
    """
    n_sub = chunk // 128
    xz = np.zeros((16, x.shape[1]), np.float32)
    xz[0:IN] = x
    xz[IN] = 1.0
    v = xz.reshape(16, -1, 128, n_sub)                    # [i, m, e, c]
    w = np.ascontiguousarray(v.transpose(0, 1, 3, 2))     # [i, m, c, e]
    return w.reshape(16, x.shape[1])


def prepare_inputs_v2(hidden, x, P, b_v, K, C, P_z, b_z, e_e, e_i, n_cores=N_CORES,
                      chunk=8192, pre_inject=False, evict_split=False):
    import ml_dtypes

    bf16 = ml_dtypes.bfloat16
    hidden = np.ascontiguousarray(np.asarray(hidden, np.float32))
    x = np.asarray(x, np.float32)
    rhs128 = _build_rhs(
        np.asarray(P), np.asarray(b_v), np.asarray(K), np.asarray(C),
        np.asarray(P_z), np.asarray(b_z), np.asarray(e_e), np.asarray(e_i),
    )
    blk = rhs128[0:64, 0:96].astype(bf16)                 # (64, 96)
    if evict_split:
        # pad to (64, 128): z cols 0:48, u cols 64:112 (strip-aligned
        # psum rows for the split eviction)
        blk128 = np.zeros((64, 128), blk.dtype)
        blk128[:, 0:48] = blk[:, 0:48]
        blk128[:, 64:112] = blk[:, 48:96]
        blk = blk128
    if pre_inject:
        # batch-major [B, 16] = [x; ones; zeros].T
        xz = np.zeros((16, x.shape[1]), np.float32)
        xz[0:IN] = x
        xz[IN] = 1.0
        xpad = np.ascontiguousarray(xz.T.astype(bf16))
    else:
        xpad = _build_xpad_v2(x, chunk).astype(bf16)
    b_core = hidden.shape[0] // n_cores
    in_maps = []
    for k in range(n_cores):
        s = slice(k * b_core, (k + 1) * b_core)
        in_maps.append(
            {
                "hidden": hidden[s],
                "xpad": np.ascontiguousarray(xpad[s] if pre_inject
                                             else xpad[:, s]),
                "rhsw": blk,
            }
        )
    return in_maps


def get_nc(b_core=B_CORE, reps=1, stage=5):
    key = (b_core, reps, stage)
    if key not in _NC_CACHE:
        _NC_CACHE[key] = _build_nc(b_core, reps, stage)
    return _NC_CACHE[key]


def prepare_inputs(hidden, x, P, b_v, K, C, P_z, b_z, e_e, e_i, n_cores=N_CORES):
    """Host-side prep: returns per-core in_maps."""
    import ml_dtypes

    bf16 = ml_dtypes.bfloat16
    hidden = np.ascontiguousarray(np.asarray(hidden, np.float32))
    x = np.asarray(x, np.float32)
    rhs = _build_rhs(
        np.asarray(P), np.asarray(b_v), np.asarray(K), np.asarray(C),
        np.asarray(P_z), np.asarray(b_z), np.asarray(e_e), np.asarray(e_i),
    ).astype(bf16)
    xpad = _build_xpad(x).astype(bf16)
    b_core = hidden.shape[0] // n_cores
    in_maps = []
    for k in range(n_cores):
        s = slice(k * b_core, (k + 1) * b_core)
        in_maps.append(
            {
                "hidden": hidden[s],
                "xpad": np.ascontiguousarray(xpad[:, s]),
                "rhsw": rhs,
            }
        )
    return in_maps


def get_nc_v2(b_core=B_CORE, reps=1, bench=False):
    key = ("v2", b_core, reps, bench)
    if key not in _NC_CACHE:
        _NC_CACHE[key] = _build_nc_v2(b_core, reps, bench)
    return _NC_CACHE[key]


def kernel(hidden, x, P, b_v, K, C, P_z, b_z, e_e, e_i):
    from concourse.bass_utils import run_bass_kernel_spmd

    nc = get_nc_v4(B_CORE)
    in_maps = prepare_inputs_v2(hidden, x, P, b_v, K, C, P_z, b_z, e_e, e_i,
                                chunk=KCFG["chunk"],
                                pre_inject=KCFG["pre_inject"] or KCFG["v5"],
                                evict_split=KCFG["evict_split"])
    res = run_bass_kernel_spmd(nc, in_maps, list(range(N_CORES)))
    out = np.concatenate([r["out"] for r in res.results], axis=0)
    return out.astype(np.float32)

